# revision 1
# baseline (speedup 1.0000x reference)
"""Trainium2 Bass kernel for nn_Discriminator_lstm (B=4096, T=32, E=H=300, VOCAB=10000).

Strategy (data-parallel over batch, 8 cores x 512 rows):
  Host: globally sort rows by cap_len, deal ranks round-robin to cores
        (every core gets the same length distribution), sorted ascending
        within each core.  m-tile m then has max length steps[m] (~8/16/24/32),
        and the recurrence runs only steps[m] steps for tile m.
  Phase 1 (per core): G = [embT;1] ^T @ [wih;b] in bf16 -> DRAM scratch
        [10000, 1200], scaled by S=512 (bias folded in via ones-row).
  Phase 2: per live (t, m):
        - indirect-DMA gather xg = G[cap[:, t]] (bf16) into SBUF
        - PE: inject xg into PSUM (identity matmul, bf16), then accumulate
          gates += hT8^T @ waug2 as TWO fp8e4m3 DoubleRow matmuls
          (k-slots [0:2] and [2:4]; h scaled by s_h=16, W_hh by s_w=32,
          psum scale S = s_h*s_w = 512)
        - ACT: sigmoid/tanh with scale=1/S -> bf16
        - DVE: c = f*c + i*g (f32); h_bf = o*tanh(c); masked h_last capture
        - Pool: h8 = (o*s_h)*tanh(c) -> fp8 for the next step's matmul
        - PE: transpose h8 (fp8, 1 cyc/row) -> one merged DVE copy into hT8
  Phase 3: logits = h_last @ Wc^T + cls_b in bf16.
"""

import os
import sys

import numpy as np

for _p in ("/opt/trn_rl_repo", "/root/.axon_site/_ro/trn_rl_repo"):
    if os.path.isdir(_p) and _p not in sys.path:
        sys.path.insert(0, _p)

import ml_dtypes

import concourse.bass as bass
import concourse.bacc as bacc
import concourse.mybir as mybir
import concourse.tile as tile

f32 = mybir.dt.float32
bf16 = mybir.dt.bfloat16
fp8 = mybir.dt.float8e4
i32 = mybir.dt.int32
u8 = mybir.dt.uint8

np_bf16 = ml_dtypes.bfloat16
np_fp8 = ml_dtypes.float8_e4m3

B, T, V, E, H = 4096, 32, 10000, 300, 300
NCORES = 8
BC = B // NCORES          # 512 batch rows per core
M = BC // 128             # 4 m-tiles
GC = 1200                 # 4*300 gate columns
CHUNKS = [(0, 512), (512, 1024), (1024, 1200)]
E_SPLITS = [(0, 128), (128, 256), (256, 301)]   # k-tiles of the [emb|1] contraction
H_SPLITS = [(0, 128), (128, 256), (256, 300)]   # h-dim splits for transposes/classifier
VTILES = (V + 127) // 128                        # 79

S_H = 16.0                # fp8 h scale
S_W = 32.0                # fp8 W_hh scale
S = S_H * S_W             # psum gate scale


def _raw(inst):
    return getattr(inst, "ins", inst)


def build_bass(steps, upad):
    nc = bacc.Bacc("TRN2", target_bir_lowering=False, debug=False, num_devices=NCORES)

    embT_d = nc.dram_tensor("embT", [E + 1, upad], bf16, kind="ExternalInput")
    wih_d = nc.dram_tensor("wih", [E + 1, GC], bf16, kind="ExternalInput")
    waug2_d = nc.dram_tensor("waug2", [128, 4, GC], fp8, kind="ExternalInput")
    wc_d = nc.dram_tensor("wc", [H, 2], bf16, kind="ExternalInput")
    clsb_d = nc.dram_tensor("clsb", [128, 2], f32, kind="ExternalInput")
    identb_d = nc.dram_tensor("identb", [128, 128], bf16, kind="ExternalInput")
    ident8_d = nc.dram_tensor("ident8", [128, 2, 128], fp8, kind="ExternalInput")
    idx_d = nc.dram_tensor("idx", [128, T, M], i32, kind="ExternalInput")
    mask_d = nc.dram_tensor("mask", [128, T, M], u8, kind="ExternalInput")
    G_d = nc.dram_tensor("G", [upad, GC], bf16, kind="Internal")
    out_d = nc.dram_tensor("out", [BC, 2], f32, kind="ExternalOutput")

    with tile.TileContext(nc, num_cores=NCORES) as tc:
        with (
            tc.tile_pool(name="const", bufs=1) as cpool,
            tc.tile_pool(name="state", bufs=1) as spool,
            tc.tile_pool(name="psum", bufs=2, space="PSUM") as ppool,
        ):
            # ---------- constants ----------
            identb = cpool.tile([128, 128], bf16, tag="identb")
            nc.sync.dma_start(out=identb[:, :], in_=identb_d[:, :])
            ident8 = cpool.tile([128, 2, 128], fp8, tag="ident8")
            nc.sync.dma_start(out=ident8[:, :, :], in_=ident8_d[:, :, :])
            waug2 = cpool.tile([128, 4, GC], fp8, tag="waug2")
            nc.sync.dma_start(out=waug2[:, :, :], in_=waug2_d[:, :, :])
            wc_sb = []
            for k, (d0, d1) in enumerate(H_SPLITS):
                t_ = cpool.tile([d1 - d0, 2], bf16, tag=f"wc{k}")
                nc.sync.dma_start(out=t_[:, :], in_=wc_d[d0:d1, :])
                wc_sb.append(t_)
            clsb = cpool.tile([128, 2], f32, tag="clsb")
            nc.sync.dma_start(out=clsb[:, :], in_=clsb_d[:, :])
            idx_sb = cpool.tile([128, T, M], i32, tag="idx")
            nc.sync.dma_start(out=idx_sb[:, :, :], in_=idx_d[:, :, :])
            mask_sb = cpool.tile([128, T, M], u8, tag="mask")
            nc.sync.dma_start(out=mask_sb[:, :, :], in_=mask_d[:, :, :])

            # ---------- state ----------
            # hT8 slots: [0]=h rows 0:128, [1]=128:256, [2]=256:300 (+garbage,
            # nulled by zero rows in waug2), [3]=zeros in waug2 -> don't care.
            hT8 = spool.tile([128, 4, BC], fp8, tag="hT8")
            nc.gpsimd.memset(hT8[:, :, :], 0.0)
            c_sb = spool.tile([128, M, H], bf16, tag="c")
            nc.gpsimd.memset(c_sb[:, :, :], 0.0)
            h_last = spool.tile([128, M, H], bf16, tag="hlast")
            hlT = []
            for k, (d0, d1) in enumerate(H_SPLITS):
                t_ = spool.tile([d1 - d0, BC], bf16, tag=f"hlT{k}")
                hlT.append(t_)

            # ---------- phase 1: G = [embT;1]^T @ [wih;b], scaled by S ----------
            g_stores = []
            with tc.tile_pool(name="gphase", bufs=1) as gpool, \
                 tc.tile_pool(name="gsbp", bufs=12) as gsbp:
                wih_sb = []
                for k, (d0, d1) in enumerate(E_SPLITS):
                    t_ = gpool.tile([d1 - d0, GC], bf16, tag=f"wih{k}")
                    nc.sync.dma_start(out=t_[:, :], in_=wih_d[d0:d1, :])
                    wih_sb.append(t_)
                embT_sb = []
                for k, (d0, d1) in enumerate(E_SPLITS):
                    t_ = gpool.tile([d1 - d0, upad], bf16, tag=f"emb{k}")
                    embT_sb.append(t_)
                # column-major load order so early vtiles unblock quickly
                NQ = 8
                for q in range(NQ):
                    q0, q1 = q * (upad // NQ), (q + 1) * (upad // NQ)
                    for k, (d0, d1) in enumerate(E_SPLITS):
                        nc.sync.dma_start(
                            out=embT_sb[k][:, q0:q1],
                            in_=embT_d[d0:d1, q0:q1])

                for v in range(upad // 128):
                    rows = 128
                    gps = ppool.tile([128, GC], f32, tag="gates")
                    for k, (d0, d1) in enumerate(E_SPLITS):
                        for (c0, c1) in CHUNKS:
                            nc.tensor.matmul(
                                gps[0:rows, c0:c1],
                                lhsT=embT_sb[k][:, v * 128:v * 128 + rows],
                                rhs=wih_sb[k][:, c0:c1],
                                start=(k == 0),
                                stop=(k == 2),
                            )
                    gsb = gsbp.tile([128, GC], bf16, tag="gsb")
                    nc.scalar.activation(
                        gsb[0:rows, 0:600], gps[0:rows, 0:600],
                        mybir.ActivationFunctionType.Copy, scale=S)
                    nc.vector.tensor_scalar(
                        out=gsb[0:rows, 600:1200], in0=gps[0:rows, 600:1200],
                        scalar1=S, scalar2=None, op0=mybir.AluOpType.mult)
                    st = nc.sync.dma_start(
                        out=G_d[v * 128:v * 128 + rows, :], in_=gsb[0:rows, :]
                    )
                    g_stores.append(st)

            g_done = nc.gpsimd.nop()
            for st in g_stores:
                tile.add_dep_helper(_raw(g_done), _raw(st), reason="G stored")

            # ---------- phase 2: recurrence ----------
            with tc.tile_pool(name="work", bufs=2) as wpool:
                for t in range(T):
                    ms = [m for m in range(M) if steps[m] > t]
                    xg = wpool.tile([128, M, GC], bf16, tag="xg")
                    for m in ms:
                        gather = nc.gpsimd.indirect_dma_start(
                            out=xg[:, m, :],
                            out_offset=None,
                            in_=G_d[:, :],
                            in_offset=bass.IndirectOffsetOnAxis(
                                ap=idx_sb[:, t, m:m + 1], axis=0),
                        )
                        tile.add_dep_helper(_raw(gather), _raw(g_done),
                                            reason="gather after G")

                    ifo = wpool.tile([128, M, 900], bf16, tag="ifo")
                    gt = wpool.tile([128, M, 300], bf16, tag="gt")
                    tch = wpool.tile([128, M, H], bf16, tag="tch")
                    hbf = wpool.tile([128, M, H], bf16, tag="hbf")
                    t1 = wpool.tile([128, M, H], bf16, tag="t1")
                    for m in ms:
                        gps = ppool.tile([128, GC], f32, tag="gates")
                        # inject xg (clears + seeds accumulation per chunk)
                        for (c0, c1) in CHUNKS:
                            nc.tensor.matmul(
                                gps[:, c0:c1],
                                lhsT=identb[:, :],
                                rhs=xg[:, m, c0:c1],
                                start=True,
                                stop=(t == 0),
                            )
                        if t > 0:
                            # gates += hT8^T @ waug2, fp8 DoubleRow (2 k-slot pairs)
                            for j, (s0, s1) in enumerate(((0, 2), (2, 4))):
                                for (c0, c1) in CHUNKS:
                                    nc.tensor.matmul(
                                        gps[:, c0:c1],
                                        lhsT=hT8[:, s0:s1, m * 128:(m + 1) * 128],
                                        rhs=waug2[:, s0:s1, c0:c1],
                                        start=False,
                                        stop=(j == 1),
                                        perf_mode=mybir.MatmulPerfMode.DoubleRow,
                                    )
                        nc.scalar.activation(
                            ifo[:, m, 0:600], gps[:, 0:600],
                            mybir.ActivationFunctionType.Sigmoid, scale=1.0 / S)
                        nc.scalar.activation(
                            gt[:, m, :], gps[:, 900:1200],
                            mybir.ActivationFunctionType.Tanh, scale=1.0 / S)
                        nc.scalar.activation(
                            ifo[:, m, 600:900], gps[:, 600:900],
                            mybir.ActivationFunctionType.Sigmoid, scale=1.0 / S)
                        # c = f*c + i*g ; h = o*tanh(c)
                        nc.vector.tensor_tensor(
                            out=t1[:, m, :], in0=ifo[:, m, 300:600],
                            in1=c_sb[:, m, :], op=mybir.AluOpType.mult)
                        nc.vector.tensor_tensor(
                            out=c_sb[:, m, :], in0=ifo[:, m, 0:300],
                            in1=gt[:, m, :], op=mybir.AluOpType.mult)
                        nc.vector.tensor_tensor(
                            out=c_sb[:, m, :], in0=c_sb[:, m, :],
                            in1=t1[:, m, :], op=mybir.AluOpType.add)
                        nc.scalar.activation(
                            tch[:, m, :], c_sb[:, m, :],
                            mybir.ActivationFunctionType.Tanh)
                        nc.vector.tensor_tensor(
                            out=hbf[:, m, :], in0=ifo[:, m, 600:900],
                            in1=tch[:, m, :], op=mybir.AluOpType.mult)
                        nc.vector.copy_predicated(
                            out=h_last[:, m, :],
                            mask=mask_sb[:, t, m:m + 1].to_broadcast([128, H]),
                            data=hbf[:, m, :])
                    for m in ms:
                        if t + 1 < steps[m]:
                            trp = ppool.tile([128, 3, 128], bf16, tag="tr")
                            for k, (d0, d1) in enumerate(H_SPLITS):
                                dk = d1 - d0
                                nc.tensor.transpose(
                                    out=trp[0:dk, k, :],
                                    in_=hbf[:, m, d0:d1],
                                    identity=identb[:, :])
                            # scaled fp8 convert during the psum->sbuf drain
                            nc.vector.tensor_scalar(
                                out=hT8[:, 0:2, m * 128:(m + 1) * 128],
                                in0=trp[:, 0:2, :], scalar1=S_H, scalar2=None,
                                op0=mybir.AluOpType.mult)
                            nc.vector.tensor_scalar(
                                out=hT8[0:44, 2, m * 128:(m + 1) * 128],
                                in0=trp[0:44, 2, :], scalar1=S_H, scalar2=None,
                                op0=mybir.AluOpType.mult)

                # ---------- phase 3: logits ----------
                trps = []
                for k in range(3):
                    trp = ppool.tile([128, 512], bf16, tag="gates")
                    trps.append(trp)
                for m in range(M):
                    for k, (d0, d1) in enumerate(H_SPLITS):
                        dk = d1 - d0
                        nc.tensor.transpose(
                            out=trps[k][0:dk, m * 128:(m + 1) * 128],
                            in_=h_last[:, m, d0:d1],
                            identity=identb[:, :])
                for k, (d0, d1) in enumerate(H_SPLITS):
                    dk = d1 - d0
                    nc.vector.tensor_copy(hlT[k][0:dk, :], trps[k][0:dk, :])

                lsb = wpool.tile([128, M, 2], f32, tag="lsb")
                for m in range(M):
                    lp = ppool.tile([128, 2], f32, tag="tr")
                    for k, (d0, d1) in enumerate(H_SPLITS):
                        nc.tensor.matmul(
                            lp[:, :],
                            lhsT=hlT[k][:, m * 128:(m + 1) * 128],
                            rhs=wc_sb[k][:, :],
                            start=(k == 0),
                            stop=(k == 2),
                        )
                    nc.vector.scalar_tensor_tensor(
                        out=lsb[:, m, :], in0=lp[:, :], scalar=1.0,
                        in1=clsb[:, :],
                        op0=mybir.AluOpType.mult, op1=mybir.AluOpType.add)
                nc.sync.dma_start(
                    out=out_d[:, :].rearrange("(m p) c -> p m c", p=128),
                    in_=lsb[:, :, :])

    nc.compile()
    return nc


_NC_CACHE = {}
LAST_RESULT = None


def _host_prep(inputs):
    cap = np.asarray(inputs["cap"]).astype(np.int64)
    cap_len = np.asarray(inputs["cap_len"]).astype(np.int64)
    embed_w = np.asarray(inputs["embed_w"], dtype=np.float32)
    W_ih = np.asarray(inputs["W_ih"], dtype=np.float32)
    W_hh = np.asarray(inputs["W_hh"], dtype=np.float32)
    b = (np.asarray(inputs["b_ih"], dtype=np.float32)
         + np.asarray(inputs["b_hh"], dtype=np.float32))
    cls_v = np.asarray(inputs["cls_v"], dtype=np.float32)
    cls_g = np.asarray(inputs["cls_g"], dtype=np.float32)
    cls_b = np.asarray(inputs["cls_b"], dtype=np.float32)

    # gate order [i f o g]
    perm = np.concatenate([np.arange(0, 300), np.arange(300, 600),
                           np.arange(900, 1200), np.arange(600, 900)])
    wih_aug = np.zeros((E + 1, GC), np.float32)
    wih_aug[:E] = W_ih[perm].T
    wih_aug[E] = b[perm]
    embT_aug = np.ones((E + 1, V), np.float32)
    embT_aug[:E] = embed_w.T

    Wp = W_hh[perm].T * S_W                          # [300, 1200], scaled
    waug2 = np.zeros((128, 4, GC), np.float32)
    waug2[:, 0, :] = Wp[0:128]
    waug2[:, 1, :] = Wp[128:256]
    waug2[0:44, 2, :] = Wp[256:300]

    Wc = cls_g * cls_v / np.linalg.norm(cls_v, axis=1, keepdims=True)  # [2, 300]

    # global sort by length; deal round-robin to cores
    order = np.argsort(cap_len, kind="stable")
    steps = []
    for m in range(M):
        mx = 0
        for c in range(NCORES):
            sel = order[c::NCORES]
            mx = max(mx, int(cap_len[sel[m * 128:(m + 1) * 128]].max()))
        steps.append(mx)
    steps = tuple(steps)

    # per-core used-vocab compaction
    used_l, inv_l = [], []
    for c in range(NCORES):
        sel = order[c::NCORES]
        used = np.unique(cap[sel])
        inv = np.zeros(V, np.int64)
        inv[used] = np.arange(len(used))
        used_l.append(used)
        inv_l.append(inv)
    upad = -(-max(len(u) for u in used_l) // 512) * 512

    shared = {
        "identb": np.eye(128, dtype=np.float32).astype(np_bf16),
        "ident8": np.broadcast_to(
            np.eye(128, dtype=np.float32)[:, None, :], (128, 2, 128)
        ).astype(np_fp8),
        "wih": wih_aug.astype(np_bf16),
        "waug2": waug2.astype(np_fp8),
        "wc": Wc.T.astype(np_bf16),
        "clsb": np.tile(cls_b.reshape(1, 2), (128, 1)).astype(np.float32),
    }
    in_maps = []
    for core in range(NCORES):
        sel = order[core::NCORES]
        capc = inv_l[core][cap[sel]]                   # [512, 32] remapped
        lenc = cap_len[sel]                            # [512]
        cols = np.zeros(upad, np.int64)
        cols[:len(used_l[core])] = used_l[core]
        embT_c = embT_aug[:, cols].astype(np_bf16)
        idx = np.ascontiguousarray(
            capc.reshape(M, 128, T).transpose(1, 2, 0)).astype(np.int32)
        lm = lenc.reshape(M, 128).T                    # [128, M]
        mask = (lm[:, None, :] - 1 == np.arange(T)[None, :, None]).astype(np.uint8)
        in_maps.append(dict(shared, idx=idx, embT=embT_c,
                            mask=np.ascontiguousarray(mask)))
    return in_maps, order, steps, upad


def kernel(**inputs) -> np.ndarray:
    global LAST_RESULT
    from concourse.bass_utils import run_bass_kernel_spmd
    in_maps, order, steps, upad = _host_prep(inputs)
    if (steps, upad) not in _NC_CACHE:
        _NC_CACHE[(steps, upad)] = build_bass(steps, upad)
    nc = _NC_CACHE[(steps, upad)]
    trace = bool(int(os.environ.get("KERNEL_TRACE", "0")))
    res = run_bass_kernel_spmd(nc, in_maps, core_ids=list(range(NCORES)), trace=trace)
    LAST_RESULT = res
    out = np.empty((B, 2), np.float32)
    for core in range(NCORES):
        out[order[core::NCORES]] = res.results[core]["out"].astype(np.float32)
    return out



# revision 2
# speedup vs baseline: 1.3687x; 1.3687x over previous
"""Trainium2 Bass kernel v2 for nn_Discriminator_lstm (B=4096, T=32, E=H=300).

Data-parallel over batch, 8 cores x 512 rows.  Host sorts rows globally by
cap_len and deals round-robin so all cores share one step profile
steps=(s0..s3) (~8/16/24/32); the recurrence runs only steps[m] steps for
m-tile m.

vs v1 (baseline):
  - G = [embT;1]^T @ [wih;b] over LIVE tokens only (dead positions remap to
    compacted index 0), compacted vocab ordered by FIRST-USE step so G
    vtiles stream in consumption order.  A prologue burst fills the first
    ~vt_need[6] vtiles through the main psum pool; the rest stream through
    a dedicated 1-bank psum pool, one chunk per tile-slot, interleaved with
    the recurrence (no serial G phase).
  - fp16 intermediates end-to-end (G, xg, gates, c, h): better precision
    than bf16 and same 2x DVE throughput.
  - g-gate 2x bake: G/Whh g-columns pre-doubled.  While >=3 tiles live
    (ACT-bound) one 1200-col sigmoid yields i,f,o and (tanh(g)+1)/2, with
    the affine fixup on DVE.  While <=2 tiles live (chain-bound) the g
    columns go through native tanh at scale 1/(2S) instead.
  - per-tile classifier emitted as soon as a tile dies (off the tail).
"""

import os
import sys

import numpy as np

for _p in ("/opt/trn_rl_repo", "/root/.axon_site/_ro/trn_rl_repo"):
    if os.path.isdir(_p) and _p not in sys.path:
        sys.path.insert(0, _p)

import ml_dtypes

import concourse.bass as bass
import concourse.bacc as bacc
import concourse.mybir as mybir
import concourse.tile as tile

f32 = mybir.dt.float32
bf16 = mybir.dt.bfloat16
f16 = mybir.dt.float16
fp8 = mybir.dt.float8e4
i32 = mybir.dt.int32
u8 = mybir.dt.uint8

np_bf16 = ml_dtypes.bfloat16
np_f16 = np.float16
np_fp8 = ml_dtypes.float8_e4m3

B, T, V, E, H = 4096, 32, 10000, 300, 300
NCORES = 8
BC = B // NCORES          # 512 rows per core
M = BC // 128             # 4 m-tiles
GC = 1200                 # 4*300 gate columns
RCH = [(0, 512), (512, 1024), (1024, 1200)]      # psum matmul chunks
H_SPLITS = [(0, 128), (128, 256), (256, 300)]

S_E = 64.0                # fp8 scale on emb
S_WI = 32.0               # fp8 scale on W_ih (+bias row)
S = S_E * S_WI            # = 2048, psum gate scale
S_H = 16.0                # fp8 h scale
S_W2 = S / S_H            # = 128, fp8 W_hh scale

LOOKAHEAD = 4             # G vtile production lookahead (steps)
PRO_STEP = 2              # prologue burst covers vt_need[PRO_STEP]
EMB_STEP = 4              # early emb chunks cover vt_need[EMB_STEP]
XPROJ = os.environ.get("XPROJ", "mixed")  # fp8 | bf16 | mixed
XPROJ_FP8 = XPROJ == "fp8"

DR = mybir.MatmulPerfMode.DoubleRow
E_SPLITS = [(0, 128), (128, 256), (256, 301)]    # bf16 x-proj k-tiles
# mixed-mode G chunks: matmul psum writes must stay within a 512-f32 bank
GQCH = [(0, 512), (512, 900), (900, 1024), (1024, 1200)]


def _raw(inst):
    return getattr(inst, "ins", inst)


def build_bass(steps, VT, vt_need, wlo, whi):
    upad = VT * 128
    nc = bacc.Bacc("TRN2", target_bir_lowering=False, debug=False,
                   num_devices=NCORES)

    if XPROJ == "fp8":
        embT8_d = nc.dram_tensor("embT8", [128, 4, upad], fp8,
                                 kind="ExternalInput")
        wih8_d = nc.dram_tensor("wih8", [128, 4, GC], fp8,
                                kind="ExternalInput")
    elif XPROJ == "bf16":
        embT8_d = nc.dram_tensor("embT8", [128, 3, upad], bf16,
                                 kind="ExternalInput")
        wih8_d = nc.dram_tensor("wih8", [128, 3, GC], bf16,
                                kind="ExternalInput")
    else:  # mixed: fp8 for i,f,o cols; bf16 for g cols
        embT8_d = nc.dram_tensor("embT8", [128, 4, upad], fp8,
                                 kind="ExternalInput")
        wih8_d = nc.dram_tensor("wih8", [128, 4, 900], fp8,
                                kind="ExternalInput")
        embTg_d = nc.dram_tensor("embTg", [128, 3, upad], bf16,
                                 kind="ExternalInput")
        wihg_d = nc.dram_tensor("wihg", [128, 3, 300], bf16,
                                kind="ExternalInput")
    waug_d = nc.dram_tensor("waug", [128, 4, GC], fp8, kind="ExternalInput")
    wc_d = nc.dram_tensor("wc", [H, 2], f16, kind="ExternalInput")
    clsb_d = nc.dram_tensor("clsb", [128, 2], f32, kind="ExternalInput")
    identb_d = nc.dram_tensor("identb", [128, 2, 128], f16,
                              kind="ExternalInput")
    idx_d = nc.dram_tensor("idx", [128, T, M], i32, kind="ExternalInput")
    mask_d = nc.dram_tensor("mask", [128, T, M], u8, kind="ExternalInput")
    G_d = nc.dram_tensor("G", [upad, GC], f16, kind="Internal")
    out_d = nc.dram_tensor("out", [BC, 2], f32, kind="ExternalOutput")

    with tile.TileContext(nc, num_cores=NCORES) as tc:
        with (
            tc.tile_pool(name="const", bufs=1) as cpool,
            tc.tile_pool(name="state", bufs=1) as spool,
            tc.tile_pool(name="work", bufs=2) as wpool,
            tc.tile_pool(name="xgp", bufs=3) as xgpool,
            tc.tile_pool(name="gsb", bufs=5) as gsbp,
            tc.tile_pool(name="pgates", bufs=2, space="PSUM") as ppool,
            tc.tile_pool(name="ptr", bufs=1, space="PSUM") as trpool,
            tc.tile_pool(name="pgq", bufs=1, space="PSUM") as gqpool,
        ):
            # ---------- constants ----------
            # identb slot 0: fp16 identity (injects / transposes).
            identc = cpool.tile([128, 2, 128], f16, tag="identb")
            nc.sync.dma_start(out=identc[:, :, :], in_=identb_d[:, :, :])
            identb = identc[:, 0, :]
            waug = cpool.tile([128, 4, GC], fp8, tag="waug")
            nc.gpsimd.dma_start(out=waug[:, :, :], in_=waug_d[:, :, :])
            if XPROJ == "fp8":
                wih8 = cpool.tile([128, 4, GC], fp8, tag="wih8")
            elif XPROJ == "bf16":
                wih8 = cpool.tile([128, 3, GC], bf16, tag="wih8")
            else:
                wih8 = cpool.tile([128, 4, 900], fp8, tag="wih8")
                wihg = cpool.tile([128, 3, 300], bf16, tag="wihg")
                nc.gpsimd.dma_start(out=wihg[:, :, :], in_=wihg_d[:, :, :])
            nc.gpsimd.dma_start(out=wih8[:, :, :], in_=wih8_d[:, :, :])
            idx_sb = cpool.tile([128, T, M], i32, tag="idx")
            nc.sync.dma_start(out=idx_sb[:, :, :], in_=idx_d[:, :, :])
            mask_sb = cpool.tile([128, T, M], u8, tag="mask")
            nc.sync.dma_start(out=mask_sb[:, :, :], in_=mask_d[:, :, :])
            wc_sb = []
            for k, (d0, d1) in enumerate(H_SPLITS):
                t_ = cpool.tile([d1 - d0, 2], f16, tag=f"wc{k}")
                nc.sync.dma_start(out=t_[:, :], in_=wc_d[d0:d1, :])
                wc_sb.append(t_)
            clsb = cpool.tile([128, 2], f32, tag="clsb")
            nc.sync.dma_start(out=clsb[:, :], in_=clsb_d[:, :])

            # embT loaded in column chunks so early vtiles unblock fast
            if XPROJ == "fp8":
                embT8 = cpool.tile([128, 4, upad], fp8, tag="embT8")
            elif XPROJ == "bf16":
                embT8 = cpool.tile([128, 3, upad], bf16, tag="embT8")
            else:
                embT8 = cpool.tile([128, 4, upad], fp8, tag="embT8")
                embTg = cpool.tile([128, 3, upad], bf16, tag="embTg")
            NQ = max(1, min(8, VT))
            qb = [(q * VT // NQ) * 128 for q in range(NQ)] + [upad]

            def load_emb(q, late):
                if qb[q + 1] <= qb[q]:
                    return
                e8 = nc.sync if late else nc.scalar
                eg = (nc.gpsimd if q % 2 else nc.sync) if late else nc.gpsimd
                e8.dma_start(out=embT8[:, :, qb[q]:qb[q + 1]],
                             in_=embT8_d[:, :, qb[q]:qb[q + 1]])
                if XPROJ == "mixed":
                    eg.dma_start(out=embTg[:, :, qb[q]:qb[q + 1]],
                                 in_=embTg_d[:, :, qb[q]:qb[q + 1]])

            NQ_EARLY = max(1, (vt_need[min(EMB_STEP, T - 1)] * NQ + VT - 1) // VT)
            for q in range(NQ_EARLY):
                load_emb(q, late=False)

            # ---------- state ----------
            hT8 = spool.tile([128, 4, BC], fp8, tag="hT8")
            nc.vector.memset(hT8[:, :, :], 0.0)
            c_sb = spool.tile([128, M, H], f16, tag="c")
            nc.vector.memset(c_sb[:, :, :], 0.0)
            # hbf/h_last padded to 384 cols; cols 300:384 stay zero so the
            # third 128-col transpose slice is fully defined.
            hbf = spool.tile([128, M, 384], f16, tag="hbf")
            nc.vector.memset(hbf[:, :, :], 0.0)
            h_last = spool.tile([128, M, 384], f16, tag="hlast")
            nc.vector.memset(h_last[:, :, :], 0.0)
            lsb = spool.tile([128, M, 2], f32, tag="lsb")

            # ---------- G production ----------
            # state: next (vtile, chunk) to emit; gsb staging per vtile
            gstate = {"v": 0, "k": 0, "gsb": None}
            store_inst = []

            def _g_mm(gp, off, v, c0, c1):
                w = c1 - c0
                if XPROJ == "fp8" or (XPROJ == "mixed" and c1 <= 900):
                    nc.tensor.matmul(
                        gp[:, off:off + w],
                        lhsT=embT8[:, 0:2, v * 128:(v + 1) * 128],
                        rhs=wih8[:, 0:2, c0:c1],
                        start=True, stop=False, perf_mode=DR)
                    nc.tensor.matmul(
                        gp[:, off:off + w],
                        lhsT=embT8[:, 2:4, v * 128:(v + 1) * 128],
                        rhs=wih8[:, 2:4, c0:c1],
                        start=False, stop=True, perf_mode=DR)
                elif XPROJ == "bf16":
                    for s, (d0, d1) in enumerate(E_SPLITS):
                        ks = d1 - d0
                        nc.tensor.matmul(
                            gp[:, off:off + w],
                            lhsT=embT8[0:ks, s, v * 128:(v + 1) * 128],
                            rhs=wih8[0:ks, s, c0:c1],
                            start=(s == 0), stop=(s == 2))
                else:  # mixed, g columns (900:1200) in bf16
                    for s, (d0, d1) in enumerate(E_SPLITS):
                        ks = d1 - d0
                        nc.tensor.matmul(
                            gp[:, off:off + w],
                            lhsT=embTg[0:ks, s, v * 128:(v + 1) * 128],
                            rhs=wihg[0:ks, s, c0 - 900:c1 - 900],
                            start=(s == 0), stop=(s == 2))

            def _g_drain(dst, src, par):
                if par == 0:
                    nc.scalar.activation(dst, src,
                                         mybir.ActivationFunctionType.Copy)
                else:
                    nc.vector.tensor_copy(dst, src)

            def pump_vtile_full(nv):
                """One whole vtile through the main gates pool."""
                v = gstate["v"]
                if v >= min(nv, VT) or gstate["k"] != 0:
                    return
                gsb = gsbp.tile([128, GC], f16, tag="gsb", name="gsbf")
                gp = ppool.tile([128, GC], f32, tag="gates", name="gpf")
                CH = GQCH if XPROJ == "mixed" else RCH
                for (c0, c1) in CH:
                    _g_mm(gp, c0, v, c0, c1)
                nc.scalar.activation(gsb[:, 0:900], gp[:, 0:900],
                                     mybir.ActivationFunctionType.Copy)
                nc.vector.tensor_copy(gsb[:, 900:GC], gp[:, 900:GC])
                st = nc.sync.dma_start(
                    out=G_d[v * 128:(v + 1) * 128, :], in_=gsb[:, :])
                store_inst.append(st)
                gstate["v"] = v + 1

            def pump_burst(nv):
                """Prologue: whole vtiles through the main gates pool."""
                while gstate["v"] < min(nv, VT):
                    pump_vtile_full(nv)

            def pump_chunk(target_v):
                """In-loop: ONE psum chunk through the 1-bank gq pool."""
                v, k = gstate["v"], gstate["k"]
                if v >= min(target_v, VT):
                    return
                CH = GQCH if XPROJ == "mixed" else RCH
                if k == 0:
                    gstate["gsb"] = gsbp.tile([128, GC], f16, tag="gsb",
                                              name="gsbq")
                c0, c1 = CH[k]
                gq = gqpool.tile([128, 512], f32, tag="gq")
                _g_mm(gq, 0, v, c0, c1)
                _g_drain(gstate["gsb"][:, c0:c1], gq[:, 0:c1 - c0],
                         (v * len(CH) + k) % 2)
                if k == len(CH) - 1:
                    st = nc.sync.dma_start(
                        out=G_d[v * 128:(v + 1) * 128, :],
                        in_=gstate["gsb"][:, :])
                    store_inst.append(st)
                    gstate["v"], gstate["k"] = v + 1, 0
                else:
                    gstate["k"] = k + 1

            def pump_to(target_v):
                """Force-complete vtiles below target (gather dep safety)."""
                while gstate["v"] < min(target_v, VT):
                    pump_chunk(target_v)

            # ---------- gathers ----------
            xg_of = {}

            def emit_gather(t):
                m0 = next(m for m in range(M) if steps[m] > t)
                nv = vt_need[t]
                pump_to(nv)
                xg = xgpool.tile([128, M, GC], f16, tag="xg")
                # NB: multi-index (>1 idx per partition) indirect gathers
                # return garbage on real HW; one gather per m-tile.
                for m in range(m0, M):
                    g = nc.gpsimd.indirect_dma_start(
                        out=xg[:, m, :],
                        out_offset=None,
                        in_=G_d[0:nv * 128, :],
                        in_offset=bass.IndirectOffsetOnAxis(
                            ap=idx_sb[:, t, m:m + 1], axis=0),
                    )
                    tile.add_dep_helper(_raw(g), _raw(store_inst[nv - 1]),
                                        reason="gather after G stores")
                xg_of[t] = xg

            def emit_phase3(m):
                trp = trpool.tile([128, 3, 128], f16, tag="tr")
                for k in range(3):
                    nc.tensor.transpose(
                        out=trp[:, k, :],
                        in_=h_last[:, m, k * 128:(k + 1) * 128],
                        identity=identb)
                hl = wpool.tile([128, 3, 128], f16, tag="hl")
                nc.vector.tensor_copy(hl[:, :, :], trp[:, :, :])
                lpt = ppool.tile([128, GC], f32, tag="gates")
                lp = lpt[:, 0:2]
                for k, (d0, d1) in enumerate(H_SPLITS):
                    dk = d1 - d0
                    nc.tensor.matmul(
                        lp[:, :],
                        lhsT=hl[0:dk, k, :],
                        rhs=wc_sb[k][:, :],
                        start=(k == 0), stop=(k == 2))
                nc.vector.scalar_tensor_tensor(
                    out=lsb[:, m, :], in0=lp[:, :], scalar=1.0,
                    in1=clsb[:, :],
                    op0=mybir.AluOpType.mult, op1=mybir.AluOpType.add)

            # staged prologue: just enough G for gather(0), then widen
            pump_burst(vt_need[0])
            emit_gather(0)
            pump_burst(vt_need[min(1, T - 1)])
            emit_gather(1)
            pump_burst(vt_need[min(PRO_STEP, T - 1)])
            for q in range(NQ_EARLY, NQ):
                load_emb(q, late=True)

            # ---------- recurrence ----------
            for t in range(T):
                ms = [m for m in range(M) if steps[m] > t]
                L = len(ms)
                baked = L >= 3
                xg = xg_of.pop(t)
                target = vt_need[min(t + LOOKAHEAD, T - 1)]

                ifo = wpool.tile([128, M, GC], f16, tag="ifo")
                t1 = wpool.tile([128, M, H], f16, tag="t1")
                t2 = wpool.tile([128, M, H], f16, tag="t2")
                gt = wpool.tile([128, M, H], f16, tag="gt")
                tch = wpool.tile([128, M, H], f16, tag="tch")

                for m in ms:
                    gps = ppool.tile([128, GC], f32, tag="gates")
                    for (c0, c1) in RCH:
                        nc.tensor.matmul(
                            gps[:, c0:c1],
                            lhsT=identb,
                            rhs=xg[:, m, c0:c1],
                            start=True, stop=(t == 0))
                    if t > 0:
                        for j, (s0, s1) in enumerate(((0, 2), (2, 4))):
                            for (c0, c1) in RCH:
                                nc.tensor.matmul(
                                    gps[:, c0:c1],
                                    lhsT=hT8[:, s0:s1, m * 128:(m + 1) * 128],
                                    rhs=waug[:, s0:s1, c0:c1],
                                    start=False, stop=(j == 1),
                                    perf_mode=DR)
                    pump_chunk(target)

                    if baked:
                        # one sigmoid: i,f,o and (tanh(g)+1)/2 (g-cols baked)
                        nc.scalar.activation(
                            ifo[:, m, :], gps[:, :],
                            mybir.ActivationFunctionType.Sigmoid,
                            scale=1.0 / S)
                        # c = f*c + 2*(i.gp) - i ; f*c on Pool (SBUF-only op)
                        nc.gpsimd.tensor_tensor(
                            out=t1[:, m, :], in0=ifo[:, m, 300:600],
                            in1=c_sb[:, m, :], op=mybir.AluOpType.mult)
                        nc.vector.tensor_tensor(
                            out=t2[:, m, :], in0=ifo[:, m, 900:1200],
                            in1=ifo[:, m, 0:300], op=mybir.AluOpType.mult)
                        nc.vector.scalar_tensor_tensor(
                            out=c_sb[:, m, :], in0=t2[:, m, :], scalar=2.0,
                            in1=ifo[:, m, 0:300],
                            op0=mybir.AluOpType.mult,
                            op1=mybir.AluOpType.subtract)
                        nc.vector.tensor_tensor(
                            out=c_sb[:, m, :], in0=c_sb[:, m, :],
                            in1=t1[:, m, :], op=mybir.AluOpType.add)
                    else:
                        # chain-bound: split sigmoid so i,f land early, and
                        # native tanh for g (scale undoes the bake)
                        nc.scalar.activation(
                            ifo[:, m, 0:600], gps[:, 0:600],
                            mybir.ActivationFunctionType.Sigmoid,
                            scale=1.0 / S)
                        nc.scalar.activation(
                            gt[:, m, :], gps[:, 900:1200],
                            mybir.ActivationFunctionType.Tanh,
                            scale=0.5 / S)
                        nc.scalar.activation(
                            ifo[:, m, 600:900], gps[:, 600:900],
                            mybir.ActivationFunctionType.Sigmoid,
                            scale=1.0 / S)
                        nc.vector.tensor_tensor(
                            out=t1[:, m, :], in0=ifo[:, m, 300:600],
                            in1=c_sb[:, m, :], op=mybir.AluOpType.mult)
                        nc.vector.tensor_tensor(
                            out=c_sb[:, m, :], in0=ifo[:, m, 0:300],
                            in1=gt[:, m, :], op=mybir.AluOpType.mult)
                        nc.vector.tensor_tensor(
                            out=c_sb[:, m, :], in0=c_sb[:, m, :],
                            in1=t1[:, m, :], op=mybir.AluOpType.add)
                    pump_chunk(target)
                    if L >= 3 and m == ms[1]:
                        pump_vtile_full(target)

                # gather launched mid-step: Pool is past this step's t1 ops,
                # and it completes before step t+1 needs Pool again
                if t + 2 < T:
                    emit_gather(t + 2)

                for m in ms:
                    nc.scalar.activation(
                        tch[:, m, :], c_sb[:, m, :],
                        mybir.ActivationFunctionType.Tanh)
                    nc.vector.tensor_tensor(
                        out=hbf[:, m, 0:H], in0=ifo[:, m, 600:900],
                        in1=tch[:, m, :], op=mybir.AluOpType.mult)

                    if t + 1 < steps[m]:
                        trp = trpool.tile([128, 3, 128], f16, tag="tr")
                        for k in range(3):
                            nc.tensor.transpose(
                                out=trp[:, k, :],
                                in_=hbf[:, m, k * 128:(k + 1) * 128],
                                identity=identb)
                        if L >= 3:
                            nc.vector.tensor_scalar(
                                out=hT8[:, 0:3, m * 128:(m + 1) * 128],
                                in0=trp[:, 0:3, :], scalar1=S_H, scalar2=None,
                                op0=mybir.AluOpType.mult)
                        else:
                            # split: slots 0,1 unblock the j0 DR matmul early
                            nc.vector.tensor_scalar(
                                out=hT8[:, 0:2, m * 128:(m + 1) * 128],
                                in0=trp[:, 0:2, :], scalar1=S_H, scalar2=None,
                                op0=mybir.AluOpType.mult)
                            nc.vector.tensor_scalar(
                                out=hT8[:, 2, m * 128:(m + 1) * 128],
                                in0=trp[:, 2, :], scalar1=S_H, scalar2=None,
                                op0=mybir.AluOpType.mult)
                    # capture AFTER convert: keeps it off the t+1 chain
                    if wlo[m] <= t <= whi[m]:
                        nc.vector.copy_predicated(
                            out=h_last[:, m, 0:H],
                            mask=mask_sb[:, t, m:m + 1].to_broadcast([128, H]),
                            data=hbf[:, m, 0:H])
                    if t + 1 == steps[m]:
                        emit_phase3(m)

            nc.sync.dma_start(
                out=out_d[:, :].rearrange("(m p) c -> p m c", p=128),
                in_=lsb[:, :, :])

    nc.compile()
    return nc


_NC_CACHE = {}
LAST_RESULT = None


def _host_prep(inputs):
    cap = np.asarray(inputs["cap"]).astype(np.int64)
    cap_len = np.asarray(inputs["cap_len"]).astype(np.int64)
    embed_w = np.asarray(inputs["embed_w"], dtype=np.float32)
    W_ih = np.asarray(inputs["W_ih"], dtype=np.float32)
    W_hh = np.asarray(inputs["W_hh"], dtype=np.float32)
    b = (np.asarray(inputs["b_ih"], dtype=np.float32)
         + np.asarray(inputs["b_hh"], dtype=np.float32))
    cls_v = np.asarray(inputs["cls_v"], dtype=np.float32)
    cls_g = np.asarray(inputs["cls_g"], dtype=np.float32)
    cls_b = np.asarray(inputs["cls_b"], dtype=np.float32)

    # gate order [i f o g]
    perm = np.concatenate([np.arange(0, 300), np.arange(300, 600),
                           np.arange(900, 1200), np.arange(600, 900)])
    gbake = np.ones(GC, np.float32)
    gbake[900:1200] = 2.0                       # tanh(g) = 2*sigmoid(2g)-1

    wih_aug = np.zeros((512, GC), np.float32)
    wih_aug[:E] = W_ih[perm].T
    wih_aug[E] = b[perm]
    if XPROJ == "fp8":
        wih8 = np.ascontiguousarray(
            (wih_aug * S_WI * gbake).reshape(4, 128, GC).transpose(1, 0, 2)
        ).astype(np_fp8)
        wihg = None
    elif XPROJ == "bf16":
        wih8 = np.ascontiguousarray(
            (wih_aug[:384] * S * gbake).reshape(3, 128, GC).transpose(1, 0, 2)
        ).astype(np_bf16)
        wihg = None
    else:
        wih8 = np.ascontiguousarray(
            (wih_aug[:, 0:900] * S_WI).reshape(4, 128, 900).transpose(1, 0, 2)
        ).astype(np_fp8)
        wihg = np.ascontiguousarray(
            (wih_aug[:384, 900:1200] * S * 2.0).reshape(3, 128, 300)
            .transpose(1, 0, 2)).astype(np_bf16)

    Wp = np.zeros((512, GC), np.float32)
    Wp[:H] = W_hh[perm].T
    waug = np.ascontiguousarray(
        (Wp * S_W2 * gbake).reshape(4, 128, GC).transpose(1, 0, 2)
    ).astype(np_fp8)

    Wc = cls_g * cls_v / np.linalg.norm(cls_v, axis=1, keepdims=True)

    order = np.argsort(cap_len, kind="stable")

    # shared step profile + capture windows
    steps = [0] * M
    wlo = [T] * M
    for c in range(NCORES):
        lenc = cap_len[order[c::NCORES]]
        for m in range(M):
            lm = lenc[m * 128:(m + 1) * 128]
            steps[m] = max(steps[m], int(lm.max()))
            wlo[m] = min(wlo[m], int(lm.min()) - 1)
    steps = tuple(steps)
    whi = tuple(s - 1 for s in steps)
    wlo = tuple(wlo)

    # per-core first-use-ordered live-token compaction
    used_l, cum_l, capi_l, lenc_l = [], [], [], []
    for c in range(NCORES):
        sel = order[c::NCORES]
        capc = cap[sel]                         # [512, 32]
        lenc = cap_len[sel]
        live = np.arange(T)[None, :] < lenc[:, None]
        seen = np.zeros(V, bool)
        inv = np.zeros(V, np.int64)
        used = []
        cum = []
        for t in range(T):
            toks = np.unique(capc[live[:, t], t])
            new = toks[~seen[toks]]
            inv[new] = len(used) + np.arange(len(new))
            seen[new] = True
            used.extend(new.tolist())
            cum.append(len(used))
        capi = inv[capc]
        capi[~live] = 0                          # dead positions -> row 0
        used_l.append(np.array(used, np.int64))
        cum_l.append(cum)
        capi_l.append(capi)
        lenc_l.append(lenc)

    VT = max(-(-len(u) // 128) for u in used_l)
    upad = VT * 128
    vt_need = tuple(
        max(max(1, -(-cum_l[c][t] // 128)) for c in range(NCORES))
        for t in range(T))

    ident = np.zeros((128, 2, 128), np.float32)
    ident[:, 0, :] = np.eye(128)
    shared = {
        "identb": ident.astype(np_f16),
        "wih8": wih8,
        **({"wihg": wihg} if wihg is not None else {}),
        "waug": waug,
        "wc": Wc.T.astype(np_f16),
        "clsb": np.tile(cls_b.reshape(1, 2), (128, 1)).astype(np.float32),
    }
    in_maps = []
    for c in range(NCORES):
        cols = np.zeros(upad, np.int64)
        cols[:len(used_l[c])] = used_l[c]
        eaug = np.zeros((512, upad), np.float32)
        eaug[:E] = embed_w.T[:, cols]
        eaug[E] = 1.0
        extra = {}
        if XPROJ == "bf16":
            embT8 = np.ascontiguousarray(
                eaug[:384].reshape(3, 128, upad).transpose(1, 0, 2)
            ).astype(np_bf16)
        else:
            embT8 = np.ascontiguousarray(
                (eaug * S_E).reshape(4, 128, upad).transpose(1, 0, 2)
            ).astype(np_fp8)
            if XPROJ == "mixed":
                extra["embTg"] = np.ascontiguousarray(
                    eaug[:384].reshape(3, 128, upad).transpose(1, 0, 2)
                ).astype(np_bf16)
        idx = np.ascontiguousarray(
            capi_l[c].reshape(M, 128, T).transpose(1, 2, 0)).astype(np.int32)
        lm = lenc_l[c].reshape(M, 128).T          # [128, M]
        mask = (lm[:, None, :] - 1 == np.arange(T)[None, :, None]).astype(np.uint8)
        in_maps.append(dict(shared, idx=idx, embT8=embT8,
                            mask=np.ascontiguousarray(mask), **extra))
    return in_maps, order, steps, VT, vt_need, wlo, whi


def kernel(**inputs) -> np.ndarray:
    global LAST_RESULT
    from concourse.bass_utils import run_bass_kernel_spmd
    in_maps, order, steps, VT, vt_need, wlo, whi = _host_prep(inputs)
    key = (steps, VT, vt_need, wlo, whi)
    if key not in _NC_CACHE:
        _NC_CACHE[key] = build_bass(steps, VT, vt_need, wlo, whi)
    nc = _NC_CACHE[key]
    trace = bool(int(os.environ.get("KERNEL_TRACE", "0")))
    res = run_bass_kernel_spmd(nc, in_maps, core_ids=list(range(NCORES)),
                               trace=trace)
    LAST_RESULT = res
    out = np.empty((B, 2), np.float32)
    for c in range(NCORES):
        out[order[c::NCORES]] = res.results[c]["out"].astype(np.float32)
    return out
